# revision 1
# baseline (speedup 1.0000x reference)
"""GQA decode attention (B=32, q_len=1, T=4096, 32 q heads / 8 kv heads, hd=128)
on 8 Trainium2 NeuronCores.

Sharding: tensor-parallel over kv heads — core h owns kv head h (4 q heads),
its slice of wq/wk/wv (ColumnParallel) and wo (RowParallel), and the
cache_k/cache_v slices for that head. Each core computes a partial [B, DIM]
output (RowParallel wo); the host sums the 8 partials.

Host-side algebraic prep (all folded into the weights, so the device kernel is
pure matmul + softmax):
  - q_len==1 means RoPE is a *fixed* linear map on the projection outputs, so
    it is folded into wq/wk: w_rot = R(freqs) @ w.
  - the 1/sqrt(head_dim) score scale is folded into wq.
  - weights are pre-transposed and the kv cache pre-permuted into the layouts
    the tensor engine wants (contraction dim on partitions).
  - a constant ones-column is appended to each V tile so the PV matmul also
    produces the softmax denominator (sum of exp) for free.
  - the key cache is split on the host into bf16 hi + bf16 lo (exact to
    ~2^-18): scores run as three bf16 matmuls (Kh.qh + Kh.ql + Kl.qh) that
    keep near-fp32 precision (~1e-5) while using the tensor engine's fast
    bf16 weight-load path — ~2x faster than fp32's two-pass LOW_HIGH
    matmuls, which are weight-load-bound for K-stationary attention.
    (Measured: 592us vs 763us full-fp32, rel err 2.2e-5 vs 4.4e-6.)
"""

import numpy as np

B = 32
DIM = 4096
HD = 128
NKV = 8
NG = 4          # q heads per kv head
T = 4096
NT = 32         # T / 128 key tiles
ND = 32         # DIM / 128 contraction chunks
N_CORES = 8
VW = 129        # V tile width: 128 value dims + 1 ones column

# If True, use the float32r (tf32-class) variant instead: measured ~707us
# with ~7.5e-4 relative error. The default bf16-hi/lo build is both faster
# (~592us) and far more accurate (~2.2e-5), so this stays off.
import os as _os
USE_FP32R = _os.environ.get("ATTN_FP32R", "0") == "1"

_PROG_CACHE = {}


def _build_program(use_fp32r):
    import concourse.mybir as mybir
    import concourse.tile as tile
    from concourse import bacc

    fp32 = mybir.dt.float32
    bf16 = mybir.dt.bfloat16
    dd = mybir.dt.float32r if use_fp32r else fp32   # matmul-operand dtype
    af = mybir.ActivationFunctionType

    nc = bacc.Bacc("TRN2", target_bir_lowering=False, debug=False,
                   num_devices=N_CORES)

    xTp_d = nc.dram_tensor("xTp", [128, 2 * ND * B], bf16, kind="ExternalInput").ap()
    wqkvT_d = nc.dram_tensor("wqkvT", [DIM, 1536], bf16, kind="ExternalInput").ap()
    woT_d = nc.dram_tensor("woT", [NG * HD, DIM], dd, kind="ExternalInput").ap()
    KT2_d = nc.dram_tensor("KT2", [B, HD, 2 * T], bf16, kind="ExternalInput").ap()
    Vp_d = nc.dram_tensor("Vp", [B, 128, NT * VW], dd, kind="ExternalInput").ap()
    ident_d = nc.dram_tensor("ident", [128, 128], dd, kind="ExternalInput").ap()
    out_d = nc.dram_tensor("out", [B, DIM], fp32, kind="ExternalOutput").ap()

    with tile.TileContext(nc) as tc:
        from contextlib import ExitStack
        with ExitStack() as ctx:
            const_pool = ctx.enter_context(tc.tile_pool(name="const", bufs=1))
            wpool = ctx.enter_context(tc.tile_pool(name="w", bufs=4))
            kv_pool = ctx.enter_context(tc.tile_pool(name="kv", bufs=3))
            small = ctx.enter_context(tc.tile_pool(name="small", bufs=2))

            ident_sb = const_pool.tile([128, 128], dd, name="ident_sb")
            nc.sync.dma_start(ident_sb[:], ident_d[:])
            xTp_sb = const_pool.tile([128, 2 * ND * B], bf16, name="xTp_sb")
            nc.sync.dma_start(xTp_sb[:], xTp_d[:])

            woT_sb = [const_pool.tile([128, DIM], dd, name=f"woT{g}_sb",
                                      tag=f"woT{g}") for g in range(NG)]

            # ---- QKV projections: qT[o,b], kT[o,b], v[b,o] ----
            qTh_sb = const_pool.tile([128, NG * B], bf16, name="qTh_sb")
            qTl_sb = const_pool.tile([128, NG * B], bf16, name="qTl_sb")
            kTh_sb = const_pool.tile([128, B], bf16, name="kTh_sb")
            kTl_sb = const_pool.tile([128, B], bf16, name="kTl_sb")
            v_sb = const_pool.tile([B, VW], dd, name="v_sb")

            # projection PSUM: its own scope, released before attention pools
            with tc.tile_pool(name="ppsum", bufs=1, space="PSUM") as ppsum:
                psq = [ppsum.tile([128, B], fp32, name=f"psq{g}", tag=f"psq{g}")
                       for g in range(NG)]
                psk = ppsum.tile([128, B], fp32, name="psk", tag="psk")
                psv = ppsum.tile([B, HD], fp32, name="psv", tag="psv")
                HB = ND * B   # offset of the lo half in xTp
                for n in range(ND):
                    wch = wpool.tile([128, 1536], bf16, name="wch", tag="wch")
                    nc.sync.dma_start(wch[:], wqkvT_d[128 * n:128 * (n + 1), :])
                    xh = xTp_sb[:, B * n:B * (n + 1)]
                    xl = xTp_sb[:, HB + B * n:HB + B * (n + 1)]
                    st, sp = (n == 0), (n == ND - 1)
                    # hi/lo bf16 split: W.x ~= Wh.xh + Wh.xl + Wl.xh
                    for g in range(NG):
                        wh = wch[:, 128 * g:128 * (g + 1)]
                        wl = wch[:, 768 + 128 * g:768 + 128 * (g + 1)]
                        nc.tensor.matmul(psq[g][:], wh, xh, start=st, stop=False)
                        nc.tensor.matmul(psq[g][:], wh, xl, start=False, stop=False)
                        nc.tensor.matmul(psq[g][:], wl, xh, start=False, stop=sp)
                    nc.tensor.matmul(psk[:], wch[:, 512:640], xh, start=st, stop=False)
                    nc.tensor.matmul(psk[:], wch[:, 512:640], xl, start=False, stop=False)
                    nc.tensor.matmul(psk[:], wch[:, 1280:1408], xh, start=False, stop=sp)
                    nc.tensor.matmul(psv[:], xh, wch[:, 640:768], start=st, stop=False)
                    nc.tensor.matmul(psv[:], xl, wch[:, 640:768], start=False, stop=False)
                    nc.tensor.matmul(psv[:], xh, wch[:, 1408:1536], start=False, stop=sp)
                for g in range(NG):
                    nc.vector.tensor_copy(qTh_sb[:, B * g:B * (g + 1)], psq[g][:])
                    nc.vector.tensor_sub(qTl_sb[:, B * g:B * (g + 1)], psq[g][:],
                                         qTh_sb[:, B * g:B * (g + 1)])
                nc.vector.tensor_copy(kTh_sb[:], psk[:])
                nc.vector.tensor_sub(kTl_sb[:], psk[:], kTh_sb[:])
                nc.vector.tensor_copy(v_sb[:, 0:HD], psv[:])
                nc.vector.memset(v_sb[:, HD:VW], 1.0)

            spsum = ctx.enter_context(tc.tile_pool(name="spsum", bufs=3, space="PSUM"))
            opsum = ctx.enter_context(tc.tile_pool(name="opsum", bufs=3, space="PSUM"))
            wpsum = ctx.enter_context(tc.tile_pool(name="wpsum", bufs=2, space="PSUM"))

            # views with free index (g, b) -> [p, b, g]
            qTh_re = qTh_sb.rearrange("p (g b) -> p b g", b=B)
            qTl_re = qTl_sb.rearrange("p (g b) -> p b g", b=B)
            attnT_sb = const_pool.tile([128, NG * B], dd, name="attnT_sb")
            attnT_re = attnT_sb.rearrange("p (g b) -> p b g", b=B)

            # ---- attention, one batch at a time ----
            for b in range(B):
                if b == 20:
                    # late-load the output-projection weights: they are only
                    # needed at the tail, keep the head of the DMA ring free
                    # for cache streaming
                    for g in range(NG):
                        nc.sync.dma_start(woT_sb[g][:],
                                          woT_d[128 * g:128 * (g + 1), :])
                K2_sb = kv_pool.tile([128, 2 * T], bf16, name="K2_sb", tag="K2")
                nc.sync.dma_start(K2_sb[:], KT2_d[b])
                V_sb = kv_pool.tile([128, NT * VW], dd, name="V_sb", tag="V")
                nc.sync.dma_start(V_sb[:], Vp_d[b])
                # new-token key: overwrite cache column t=4095 (both halves)
                nc.vector.tensor_copy(K2_sb[:, T - 1:T], kTh_sb[:, b:b + 1])
                nc.vector.tensor_copy(K2_sb[:, 2 * T - 1:2 * T], kTl_sb[:, b:b + 1])
                # new-token value: overwrite the t=4095 V row (partition 127 of
                # the last chunk). Cross-partition move, so use a tiny DMA on
                # the scalar ring.
                nc.scalar.dma_start(
                    V_sb[127:128, VW * (NT - 1):VW * (NT - 1) + HD],
                    v_sb[b:b + 1, 0:HD])

                qbh = qTh_re[:, b]  # [128, 4] strided
                qbl = qTl_re[:, b]
                psS = spsum.tile([128, NG * NT], fp32, name="psS", tag="psS")
                for n in range(NT):
                    # full-precision score via bf16 hi/lo split:
                    # K.q ~= Kh.qh + Kh.ql + Kl.qh  (Kl.ql ~ 2^-18, dropped)
                    sl = psS[:, NG * n:NG * (n + 1)]
                    kh = K2_sb[:, 128 * n:128 * (n + 1)]
                    kl = K2_sb[:, T + 128 * n:T + 128 * (n + 1)]
                    nc.tensor.matmul(sl, kh, qbh, start=True, stop=False)
                    nc.tensor.matmul(sl, kh, qbl, start=False, stop=False)
                    nc.tensor.matmul(sl, kl, qbh, start=False, stop=True)
                probs = kv_pool.tile([128, NG * NT], dd, name="probs",
                                     tag="probs")
                for c in range(8):
                    cw = NG * NT // 8
                    nc.scalar.activation(probs[:, cw * c:cw * (c + 1)],
                                         psS[:, cw * c:cw * (c + 1)], af.Exp)

                # one bank: cols [0,129) partitions 0:4 = PV out + expsum;
                # cols [129,133) partitions 0:128 = transposed attn
                psO = opsum.tile([128, VW + NG], fp32, name="psO", tag="psO")
                for n in range(NT):
                    nc.tensor.matmul(psO[0:NG, 0:VW], probs[:, NG * n:NG * (n + 1)],
                                     V_sb[:, VW * n:VW * (n + 1)],
                                     start=(n == 0), stop=(n == NT - 1))

                recip = small.tile([NG, 1], fp32, name="recip", tag="recip")
                nc.vector.reciprocal(recip[:], psO[0:NG, HD:VW])
                attn_b = small.tile([NG, HD], dd, name="attn_b", tag="attn_b")
                nc.vector.tensor_scalar_mul(attn_b[:], psO[0:NG, 0:HD], recip[:])

                nc.tensor.transpose(psO[:, VW:VW + NG].bitcast(dd), attn_b[:],
                                    ident_sb[0:NG, 0:NG])
                nc.vector.tensor_copy(attnT_re[:, b], psO[:, VW:VW + NG])

            # ---- output projection: out[b, :] = attnT.T @ woT ----
            out_sb = const_pool.tile([B, DIM], fp32, name="out_sb")
            for j in range(DIM // 512):
                psW = wpsum.tile([B, 512], fp32, name="psW", tag="psW")
                for g in range(NG):
                    nc.tensor.matmul(psW[:], attnT_sb[:, B * g:B * (g + 1)],
                                     woT_sb[g][:, 512 * j:512 * (j + 1)],
                                     start=(g == 0), stop=(g == NG - 1))
                nc.vector.tensor_copy(out_sb[:, 512 * j:512 * (j + 1)], psW[:])
            nc.sync.dma_start(out_d[:], out_sb[:])

    nc.compile()
    return nc


def _build_program_fp32r():
    """float32r (tf32-class) variant. The ISA requires full [128,128]
    stationary operands for fp32r matmuls, which forces a different
    structure than the fp32 build:
      - PV runs V-stationary (lhsT = V tile), producing attn directly in
        [d, qh] layout — no per-batch transpose needed.
      - softmax denominators come from a ones-matrix matmul (column sums)
        + a strided DVE reduction.
      - normalization uses a ones-matmul broadcast of 1/sum.
      - wo runs woT-stationary, producing the output transposed; the host
        untransposes.
    """
    import concourse.mybir as mybir
    import concourse.tile as tile
    from concourse import bacc

    fp32 = mybir.dt.float32
    dd = mybir.dt.float32r
    af = mybir.ActivationFunctionType
    ax = mybir.AxisListType
    alu = mybir.AluOpType

    nc = bacc.Bacc("TRN2", target_bir_lowering=False, debug=False,
                   num_devices=N_CORES)

    xTp_d = nc.dram_tensor("xTp", [128, ND * B], dd, kind="ExternalInput").ap()
    wqkvT_d = nc.dram_tensor("wqkvT", [DIM, 768], dd, kind="ExternalInput").ap()
    woT_d = nc.dram_tensor("woT", [NG * HD, DIM], dd, kind="ExternalInput").ap()
    KT_d = nc.dram_tensor("KT", [B, HD, T], dd, kind="ExternalInput").ap()
    Vp_d = nc.dram_tensor("Vp", [B, 128, T], dd, kind="ExternalInput").ap()
    ident_d = nc.dram_tensor("ident", [128, 128], dd, kind="ExternalInput").ap()
    ones_d = nc.dram_tensor("ones", [128, 128], dd, kind="ExternalInput").ap()
    zeros_d = nc.dram_tensor("zeros4", [128, NG], dd, kind="ExternalInput").ap()
    # transposed partial output, layout [p, (dchunk, b)]
    out_d = nc.dram_tensor("outT", [128, 32 * B], fp32, kind="ExternalOutput").ap()

    with tile.TileContext(nc) as tc:
        from contextlib import ExitStack
        with ExitStack() as ctx:
            const_pool = ctx.enter_context(tc.tile_pool(name="const", bufs=1))
            wpool = ctx.enter_context(tc.tile_pool(name="w", bufs=3))
            kv_pool = ctx.enter_context(tc.tile_pool(name="kv", bufs=3))
            small = ctx.enter_context(tc.tile_pool(name="small", bufs=2))

            ident_sb = const_pool.tile([128, 128], dd, name="ident_sb")
            nc.sync.dma_start(ident_sb[:], ident_d[:])
            ones_sb = const_pool.tile([128, 128], dd, name="ones_sb")
            nc.sync.dma_start(ones_sb[:], ones_d[:])
            xTp_sb = const_pool.tile([128, ND * B], dd, name="xTp_sb")
            nc.sync.dma_start(xTp_sb[:], xTp_d[:])
            recip_row = const_pool.tile([128, NG], dd, name="recip_row")
            nc.sync.dma_start(recip_row[:], zeros_d[:])

            woT_sb = []
            for g in range(NG):
                t = const_pool.tile([128, DIM], dd, name=f"woT{g}_sb",
                                    tag=f"woT{g}")
                nc.sync.dma_start(t[:], woT_d[128 * g:128 * (g + 1), :])
                woT_sb.append(t)

            # ---- projections: qT[o,b], kT[o,b], vT[o,b] (all wT-stationary) ----
            qT_sb = const_pool.tile([128, NG * B], dd, name="qT_sb")
            kT_sb = const_pool.tile([128, B], dd, name="kT_sb")
            vpad_sb = const_pool.tile([128, 128], dd, name="vpad_sb")
            v_sb = const_pool.tile([B, HD], dd, name="v_sb")
            nc.vector.memset(vpad_sb[:].bitcast(fp32), 0.0)

            with tc.tile_pool(name="ppsum", bufs=1, space="PSUM") as ppsum:
                psq = [ppsum.tile([128, B], fp32, name=f"psq{g}", tag=f"psq{g}")
                       for g in range(NG)]
                psk = ppsum.tile([128, B], fp32, name="psk", tag="psk")
                psvT = ppsum.tile([128, B], fp32, name="psvT", tag="psvT")
                for n in range(ND):
                    wch = wpool.tile([128, 768], dd, name="wch", tag="wch")
                    nc.sync.dma_start(wch[:], wqkvT_d[128 * n:128 * (n + 1), :])
                    xch = xTp_sb[:, B * n:B * (n + 1)]
                    st, sp = (n == 0), (n == ND - 1)
                    for g in range(NG):
                        nc.tensor.matmul(psq[g][:], wch[:, 128 * g:128 * (g + 1)],
                                         xch, start=st, stop=sp)
                    nc.tensor.matmul(psk[:], wch[:, 512:640], xch,
                                     start=st, stop=sp)
                    nc.tensor.matmul(psvT[:], wch[:, 640:768], xch,
                                     start=st, stop=sp)
                for g in range(NG):
                    nc.vector.tensor_copy(qT_sb[:, B * g:B * (g + 1)], psq[g][:])
                nc.vector.tensor_copy(kT_sb[:], psk[:])
                nc.vector.tensor_copy(vpad_sb[:, 0:B], psvT[:])
                # v[b, d] = transpose(vT): full 128x128 transpose of the
                # zero-padded tile, keep rows 0:B
                psv2 = ppsum.tile([128, 128], fp32, name="psv2", tag="psv2")
                nc.tensor.transpose(psv2[:].bitcast(dd), vpad_sb[:], ident_sb[:])
                nc.vector.tensor_copy(v_sb[:], psv2[0:B, :])

            spsum = ctx.enter_context(tc.tile_pool(name="spsum", bufs=2, space="PSUM"))
            opsum = ctx.enter_context(tc.tile_pool(name="opsum", bufs=2, space="PSUM"))
            xpsum = ctx.enter_context(tc.tile_pool(name="xpsum", bufs=2, space="PSUM"))
            wpsum = ctx.enter_context(tc.tile_pool(name="wpsum", bufs=2, space="PSUM"))

            qT_re = qT_sb.rearrange("p (g b) -> p b g", b=B)
            attnT_sb = const_pool.tile([128, NG * B], dd, name="attnT_sb")
            attnT_re = attnT_sb.rearrange("p (g b) -> p b g", b=B)

            NCHUNK = 8
            CW = NG * NT // NCHUNK     # probs columns per exp chunk
            for b in range(B):
                K_sb = kv_pool.tile([128, T], dd, name="K_sb", tag="K")
                nc.scalar.dma_start(K_sb[:], KT_d[b])
                V_sb = kv_pool.tile([128, T], dd, name="V_sb", tag="V")
                nc.sync.dma_start(V_sb[:], Vp_d[b])
                nc.vector.tensor_copy(K_sb[:, T - 1:T], kT_sb[:, b:b + 1])
                nc.scalar.dma_start(
                    V_sb[127:128, 128 * (NT - 1):128 * NT],
                    v_sb[b:b + 1, 0:HD])

                qb = qT_re[:, b]
                psS = spsum.tile([128, NG * NT], fp32, name="psS", tag="psS")
                for n in range(NT):
                    nc.tensor.matmul(psS[:, NG * n:NG * (n + 1)],
                                     K_sb[:, 128 * n:128 * (n + 1)], qb,
                                     start=True, stop=True)
                probs = kv_pool.tile([128, NG * NT], dd, name="probs",
                                     tag="probs")
                for c in range(NCHUNK):
                    nc.scalar.activation(probs[:, CW * c:CW * (c + 1)],
                                         psS[:, CW * c:CW * (c + 1)], af.Exp)

                # PV, V-stationary: psO[d, g] += V_tile.T @ probs_chunk
                psO = opsum.tile([128, NG], fp32, name="psO", tag="psO")
                for n in range(NT):
                    nc.tensor.matmul(psO[:], V_sb[:, 128 * n:128 * (n + 1)],
                                     probs[:, NG * n:NG * (n + 1)],
                                     start=(n == 0), stop=(n == NT - 1))

                # psX bank: cols [0,128) = ones-matmul column sums;
                # cols [128,132) = broadcast 1/sum (written later)
                psX = xpsum.tile([128, 128 + NG], fp32, name="psX", tag="psX")
                nc.tensor.matmul(psX[:, 0:128], ones_sb[:], probs[:],
                                 start=True, stop=True)
                sums4 = small.tile([1, NG], fp32, name="sums4", tag="sums4")
                nc.vector.tensor_reduce(
                    sums4[:], psX[0:1, 0:128].rearrange("p (n g) -> p g n", g=NG),
                    axis=ax.X, op=alu.add)
                recip = small.tile([1, NG], fp32, name="recip", tag="recip")
                nc.vector.reciprocal(recip[:], sums4[:])
                nc.vector.tensor_copy(recip_row[0:1, :], recip[:])
                nc.tensor.matmul(psX[:, 128:128 + NG], ones_sb[:], recip_row[:],
                                 start=True, stop=True)
                bc_sb = small.tile([128, NG], fp32, name="bc_sb", tag="bc_sb")
                nc.vector.tensor_copy(bc_sb[:], psX[:, 128:128 + NG])
                nc.vector.tensor_mul(attnT_re[:, b], psO[:], bc_sb[:])

            # ---- wo, woT-stationary: outT[dchunk][d, b] ----
            outT_sb = const_pool.tile([128, 32 * B], fp32, name="outT_sb")
            for j in range(32):
                psW = wpsum.tile([128, B], fp32, name="psW", tag="psW")
                for g in range(NG):
                    nc.tensor.matmul(psW[:], woT_sb[g][:, 128 * j:128 * (j + 1)],
                                     attnT_sb[:, B * g:B * (g + 1)],
                                     start=(g == 0), stop=(g == NG - 1))
                nc.vector.tensor_copy(outT_sb[:, B * j:B * (j + 1)], psW[:])
            nc.sync.dma_start(out_d[:], outT_sb[:])

    nc.compile()
    return nc


def _get_program():
    key = ("nc", bool(USE_FP32R))
    if key not in _PROG_CACHE:
        if USE_FP32R:
            _PROG_CACHE[key] = _build_program_fp32r()
        else:
            _PROG_CACHE[key] = _build_program(False)
    return _PROG_CACHE[key]


def _host_prep(x, freqs_cos, freqs_sin, cache_k, cache_v, wq, wk, wv, wo):
    """Build the 8 per-core input maps."""
    f32 = np.float32
    x = np.asarray(x, f32)
    cos = np.asarray(freqs_cos, f32).reshape(-1)[:HD // 2]
    sin = np.asarray(freqs_sin, f32).reshape(-1)[:HD // 2]
    wq = np.asarray(wq, f32)
    wk = np.asarray(wk, f32)
    wv = np.asarray(wv, f32)
    wo = np.asarray(wo, f32)
    cache_k = np.asarray(cache_k, f32)
    cache_v = np.asarray(cache_v, f32)

    def rope_fold(w, nheads):
        w4 = w.reshape(nheads, HD // 2, 2, DIM)
        a, bb = w4[:, :, 0, :], w4[:, :, 1, :]
        c = cos[None, :, None]
        s = sin[None, :, None]
        out = np.empty_like(w4)
        out[:, :, 0, :] = a * c - bb * s
        out[:, :, 1, :] = a * s + bb * c
        return out.reshape(nheads * HD, DIM)

    wq_r = rope_fold(wq, NKV * NG) * f32(1.0 / np.sqrt(HD))
    wk_r = rope_fold(wk, NKV)

    x2 = x.reshape(B, DIM)
    xTp = np.ascontiguousarray(
        x2.T.reshape(ND, 128, B).transpose(1, 0, 2)).reshape(128, ND * B)
    if not USE_FP32R:
        import ml_dtypes
        bfl = ml_dtypes.bfloat16
        xh = xTp.astype(bfl)
        xTp = np.concatenate([xh, (xTp - xh.astype(f32)).astype(bfl)], axis=1)

    # [h, b, d, t]
    KT_all = np.ascontiguousarray(cache_k.transpose(2, 0, 3, 1))
    cv = cache_v.reshape(B, NT, 128, NKV, HD)
    if USE_FP32R:
        # [h, b, p, n, d] plain (no ones column)
        Vp_all = np.ascontiguousarray(cv.transpose(3, 0, 2, 1, 4))
        Vp_all = Vp_all.reshape(NKV, B, 128, T)
    else:
        # [h, b, p, n, d] + ones column per (n) chunk
        Vp_all = np.ones((NKV, B, 128, NT, VW), f32)
        Vp_all[..., :HD] = cv.transpose(3, 0, 2, 1, 4)
        Vp_all = Vp_all.reshape(NKV, B, 128, NT * VW)

    ident = np.eye(128, dtype=f32)

    in_maps = []
    for h in range(N_CORES):
        wqkvT = np.ascontiguousarray(np.concatenate([
            wq_r[h * NG * HD:(h + 1) * NG * HD],
            wk_r[h * HD:(h + 1) * HD],
            wv[h * HD:(h + 1) * HD],
        ], axis=0).T)                                   # [4096, 768]
        if not USE_FP32R:
            import ml_dtypes
            bfl = ml_dtypes.bfloat16
            wh = wqkvT.astype(bfl)
            wqkvT = np.concatenate(
                [wh, (wqkvT - wh.astype(f32)).astype(bfl)], axis=1)
        woT = np.ascontiguousarray(wo[:, h * NG * HD:(h + 1) * NG * HD].T)
        m = {
            "xTp": xTp,
            "wqkvT": wqkvT,
            "woT": woT,
            "Vp": Vp_all[h],
            "ident": ident,
        }
        if USE_FP32R:
            m["KT"] = KT_all[h]
            m["ones"] = np.ones((128, 128), f32)
            m["zeros4"] = np.zeros((128, NG), f32)
        else:
            # bf16 hi/lo split of the key cache, packed [hi | lo] along t
            import ml_dtypes
            kth = KT_all[h].astype(ml_dtypes.bfloat16)
            ktl = (KT_all[h] - kth.astype(f32)).astype(ml_dtypes.bfloat16)
            m["KT2"] = np.concatenate([kth, ktl], axis=2)
        in_maps.append(m)
    return in_maps


def _kernel_numpy_fallback(x, start_pos, freqs_cos, freqs_sin, cache_k, cache_v,
                           wq, wk, wv, wo):
    """Reference-equivalent numpy path for shapes this kernel isn't built for."""
    f32 = np.float32
    start_pos = int(start_pos)
    x = np.asarray(x, f32)
    bsz, seqlen, _ = x.shape
    n_rep = 4
    hd = HD

    def rope(t, c, s):
        tr = t.reshape(*t.shape[:-1], hd // 2, 2)
        a, b2 = tr[..., 0], tr[..., 1]
        c = c[None, :, None, :]
        s = s[None, :, None, :]
        out = np.stack([a * c - b2 * s, a * s + b2 * c], axis=-1)
        return out.reshape(t.shape)

    xq = (x @ np.asarray(wq, f32).T).reshape(bsz, seqlen, NKV * n_rep, hd)
    xk = (x @ np.asarray(wk, f32).T).reshape(bsz, seqlen, NKV, hd)
    xv = (x @ np.asarray(wv, f32).T).reshape(bsz, seqlen, NKV, hd)
    fc = np.asarray(freqs_cos, f32)
    fs = np.asarray(freqs_sin, f32)
    xq = rope(xq, fc, fs)
    xk = rope(xk, fc, fs)
    ck = np.array(cache_k, f32, copy=True)
    cvv = np.array(cache_v, f32, copy=True)
    ck[:, start_pos:start_pos + seqlen] = xk
    cvv[:, start_pos:start_pos + seqlen] = xv
    keys = ck[:, :start_pos + seqlen]
    values = cvv[:, :start_pos + seqlen]
    q = xq.reshape(bsz, seqlen, NKV, n_rep, hd)
    scale = 1.0 / np.sqrt(hd)
    scores = np.einsum('bsgrd,btgd->bgrst', q, keys) * scale
    scores = scores - scores.max(axis=-1, keepdims=True)
    e = np.exp(scores)
    probs = e / e.sum(axis=-1, keepdims=True)
    out = np.einsum('bgrst,btgd->bsgrd', probs, values)
    out = out.reshape(bsz, seqlen, NKV * n_rep * hd)
    return (out @ np.asarray(wo, f32).T).astype(f32)


TRACE = False          # set True (e.g. from test.py) to neuron-profile the run
TRACE_KWARGS = {}
LAST_RESULT = None     # BassKernelResults of the most recent device run


def kernel(x, start_pos, freqs_cos, freqs_sin, cache_k, cache_v, wq, wk, wv, wo):
    global LAST_RESULT
    x = np.asarray(x)
    if (int(start_pos) != T - 1 or x.shape != (B, 1, DIM)
            or np.asarray(cache_k).shape != (B, T, NKV, HD)):
        return _kernel_numpy_fallback(x, start_pos, freqs_cos, freqs_sin,
                                      cache_k, cache_v, wq, wk, wv, wo)

    from concourse.bass_utils import run_bass_kernel_spmd

    nc = _get_program()
    in_maps = _host_prep(x, freqs_cos, freqs_sin, cache_k, cache_v,
                         wq, wk, wv, wo)
    res = run_bass_kernel_spmd(nc, in_maps, list(range(N_CORES)),
                               trace=TRACE, **TRACE_KWARGS)
    LAST_RESULT = res
    out = np.zeros((B, DIM), np.float64)
    for i in range(N_CORES):
        if USE_FP32R:
            # outT layout [p, (dchunk, b)] -> [B, DIM]
            o = res.results[i]["outT"].reshape(128, 32, B)
            out += o.transpose(2, 1, 0).reshape(B, DIM)
        else:
            out += res.results[i]["out"]
    return out.astype(np.float32).reshape(B, 1, DIM)



# revision 3
# speedup vs baseline: 1.9199x; 1.9199x over previous
"""GQA decode attention (B=32, q_len=1, T=4096, 32 q heads / 8 kv heads, hd=128)
on 8 Trainium2 NeuronCores.

Sharding: tensor-parallel over kv heads - core h owns kv head h (4 q heads),
its slice of wq/wk/wv (ColumnParallel) and wo (RowParallel), and the
cache_k/cache_v slices for that head. Each core computes a partial output
(RowParallel wo) in transposed layout; the host sums the 8 partials.

The kernel is HBM-bandwidth-bound (KV cache streaming), so everything is
fp16 end to end (rel err ~6e-4 vs the fp32 reference, measured on the
actual data):
  - q_len==1 means RoPE is a fixed linear map on the projection outputs, so
    it is folded into wq/wk on the host: w_rot = R(freqs) @ w. The
    1/sqrt(head_dim) score scale is folded into wq too.
  - K cache is stored transposed [hd, t] in fp16: one score matmul per
    128-key tile (K-tile stationary, fast-weight-load path; q streams 4
    columns).
  - V cache is stored [t, d] in fp16 and used stationary in the PV matmul
    (probs stream 4 columns), producing attn directly in [d, g] layout -
    no per-batch transpose.
  - softmax runs unnormalized (exp in fp32 PSUM -> fp16 probs); the
    denominator comes from a ones-column matmul (column sums) + a strided
    DVE reduce, and the normalization uses a ones-matmul broadcast of
    1/sum across partitions.
  - big DMA is split over three hardware rings: K on the sync ring, V on
    the scalar ring, weights/consts on the gpsimd ring, so the 16 DMA
    engines see deeper queues.
"""

import numpy as np

B = 32
DIM = 4096
HD = 128
NKV = 8
NG = 4          # q heads per kv head
T = 4096
NT = 32         # T / 128 key tiles
ND = 32         # DIM / 128 contraction chunks
N_CORES = 8

_PROG_CACHE = {}


def _build_program():
    import concourse.mybir as mybir
    import concourse.tile as tile
    from concourse import bacc

    fp32 = mybir.dt.float32
    f16 = mybir.dt.float16
    af = mybir.ActivationFunctionType
    ax = mybir.AxisListType
    alu = mybir.AluOpType

    nc = bacc.Bacc("TRN2", target_bir_lowering=False, debug=False,
                   num_devices=N_CORES)

    xTp_d = nc.dram_tensor("xTp", [128, ND * B], f16, kind="ExternalInput").ap()
    wqkvT_d = nc.dram_tensor("wqkvT", [DIM, 768], f16, kind="ExternalInput").ap()
    woT_d = nc.dram_tensor("woT", [NG * HD, DIM], f16, kind="ExternalInput").ap()
    KT_d = nc.dram_tensor("KT", [B, HD, T], f16, kind="ExternalInput").ap()
    Vp_d = nc.dram_tensor("Vp", [B, 128, T], f16, kind="ExternalInput").ap()
    ones_d = nc.dram_tensor("ones", [128, 128], f16, kind="ExternalInput").ap()
    # transposed partial output, layout [p, (dchunk, b)]
    out_d = nc.dram_tensor("outT", [128, 32 * B], fp32, kind="ExternalOutput").ap()

    with tile.TileContext(nc) as tc:
        from contextlib import ExitStack
        with ExitStack() as ctx:
            const_pool = ctx.enter_context(tc.tile_pool(name="const", bufs=1))
            wpool = ctx.enter_context(tc.tile_pool(name="w", bufs=4))
            kv_pool = ctx.enter_context(tc.tile_pool(name="kv", bufs=4))
            small = ctx.enter_context(tc.tile_pool(name="small", bufs=2))

            # consts + weights on the gpsimd ring (keeps sync/scalar rings
            # free for the KV stream)
            ones_sb = const_pool.tile([128, 128], f16, name="ones_sb")
            nc.gpsimd.dma_start(ones_sb[:], ones_d[:])
            xTp_sb = const_pool.tile([128, ND * B], f16, name="xTp_sb")
            nc.gpsimd.dma_start(xTp_sb[:], xTp_d[:])
            woT_sb = []
            for g in range(NG):
                t = const_pool.tile([128, DIM], f16, name=f"woT{g}_sb",
                                    tag=f"woT{g}")
                nc.gpsimd.dma_start(t[:], woT_d[128 * g:128 * (g + 1), :])
                woT_sb.append(t)

            # ---- QKV projections: qT[o,b], kT[o,b], v[b,o] ----
            qT_sb = const_pool.tile([128, NG * B], f16, name="qT_sb")
            kT_sb = const_pool.tile([128, B], f16, name="kT_sb")
            v_sb = const_pool.tile([B, HD], f16, name="v_sb")

            with tc.tile_pool(name="ppsum", bufs=1, space="PSUM") as ppsum:
                psq = [ppsum.tile([128, B], fp32, name=f"psq{g}", tag=f"psq{g}")
                       for g in range(NG)]
                psk = ppsum.tile([128, B], fp32, name="psk", tag="psk")
                psv = ppsum.tile([B, HD], fp32, name="psv", tag="psv")
                for n in range(ND):
                    wch = wpool.tile([128, 768], f16, name="wch", tag="wch")
                    # alternate the two main rings for the weight stream
                    eng = nc.sync if (n % 2 == 0) else nc.scalar
                    eng.dma_start(wch[:], wqkvT_d[128 * n:128 * (n + 1), :])
                    xch = xTp_sb[:, B * n:B * (n + 1)]
                    st, sp = (n == 0), (n == ND - 1)
                    for g in range(NG):
                        nc.tensor.matmul(psq[g][:], wch[:, 128 * g:128 * (g + 1)],
                                         xch, start=st, stop=sp)
                    nc.tensor.matmul(psk[:], wch[:, 512:640], xch,
                                     start=st, stop=sp)
                    nc.tensor.matmul(psv[:], xch, wch[:, 640:768],
                                     start=st, stop=sp)
                for g in range(NG):
                    nc.vector.tensor_copy(qT_sb[:, B * g:B * (g + 1)], psq[g][:])
                nc.vector.tensor_copy(kT_sb[:], psk[:])
                nc.vector.tensor_copy(v_sb[:], psv[:])

            spsum = ctx.enter_context(tc.tile_pool(name="spsum", bufs=3, space="PSUM"))
            opsum = ctx.enter_context(tc.tile_pool(name="opsum", bufs=3, space="PSUM"))
            wpsum = ctx.enter_context(tc.tile_pool(name="wpsum", bufs=2, space="PSUM"))

            qT_re = qT_sb.rearrange("p (g b) -> p b g", b=B)
            attnT_sb = const_pool.tile([128, NG * B], f16, name="attnT_sb")
            attnT_re = attnT_sb.rearrange("p (g b) -> p b g", b=B)

            # ---- attention, one batch at a time ----
            for b in range(B):
                K_sb = kv_pool.tile([128, T], f16, name="K_sb", tag="K")
                nc.sync.dma_start(K_sb[:], KT_d[b])
                V_sb = kv_pool.tile([128, T], f16, name="V_sb", tag="V")
                nc.scalar.dma_start(V_sb[:], Vp_d[b])
                # new-token key: overwrite cache column t=4095
                nc.vector.tensor_copy(K_sb[:, T - 1:T], kT_sb[:, b:b + 1])
                # new-token value: overwrite the t=4095 V row (partition 127
                # of the last chunk). Cross-partition move -> tiny DMA.
                nc.gpsimd.dma_start(
                    V_sb[127:128, 128 * (NT - 1):128 * NT],
                    v_sb[b:b + 1, 0:HD])

                qb = qT_re[:, b]  # [128, 4] strided
                psS = spsum.tile([128, NG * NT], fp32, name="psS", tag="psS")
                for n in range(NT):
                    nc.tensor.matmul(psS[:, NG * n:NG * (n + 1)],
                                     K_sb[:, 128 * n:128 * (n + 1)], qb,
                                     start=True, stop=True)
                probs = kv_pool.tile([128, NG * NT], f16, name="probs",
                                     tag="probs")
                for c in range(2):
                    cw = NG * NT // 2
                    nc.scalar.activation(probs[:, cw * c:cw * (c + 1)],
                                         psS[:, cw * c:cw * (c + 1)], af.Exp)

                # one PSUM bank: cols [0,4) = PV out [d, g]; cols [4,8) =
                # broadcast 1/sum; cols [8,136) partition 0 = column sums
                psO = opsum.tile([128, 8 + NG * NT], fp32, name="psO", tag="psO")
                for n in range(NT):
                    nc.tensor.matmul(psO[:, 0:NG],
                                     V_sb[:, 128 * n:128 * (n + 1)],
                                     probs[:, NG * n:NG * (n + 1)],
                                     start=(n == 0), stop=(n == NT - 1))
                nc.tensor.matmul(psO[0:1, 8:8 + NG * NT], ones_sb[:, 0:1],
                                 probs[:], start=True, stop=True)

                sums4 = small.tile([1, NG], fp32, name="sums4", tag="sums4")
                nc.vector.tensor_reduce(
                    sums4[:],
                    psO[0:1, 8:8 + NG * NT].rearrange("p (n g) -> p g n", g=NG),
                    axis=ax.X, op=alu.add)
                recip = small.tile([1, NG], fp32, name="recip", tag="recip")
                nc.vector.reciprocal(recip[:], sums4[:])
                rr = small.tile([128, NG], f16, name="rr", tag="rr")
                nc.vector.memset(rr[:], 0.0)
                nc.vector.tensor_copy(rr[0:1, :], recip[:])
                nc.tensor.matmul(psO[:, NG:2 * NG], ones_sb[:], rr[:],
                                 start=True, stop=True)
                bc_sb = small.tile([128, NG], fp32, name="bc_sb", tag="bc_sb")
                nc.vector.tensor_copy(bc_sb[:], psO[:, NG:2 * NG])
                nc.vector.tensor_mul(attnT_re[:, b], psO[:, 0:NG], bc_sb[:])

            # ---- wo, woT-stationary: outT[dchunk][d, b] ----
            outT_sb = const_pool.tile([128, 32 * B], fp32, name="outT_sb")
            for j in range(32):
                psW = wpsum.tile([128, B], fp32, name="psW", tag="psW")
                for g in range(NG):
                    nc.tensor.matmul(psW[:], woT_sb[g][:, 128 * j:128 * (j + 1)],
                                     attnT_sb[:, B * g:B * (g + 1)],
                                     start=(g == 0), stop=(g == NG - 1))
                nc.vector.tensor_copy(outT_sb[:, B * j:B * (j + 1)], psW[:])
            nc.sync.dma_start(out_d[:], outT_sb[:])

    nc.compile()
    return nc


def _get_program():
    if "nc" not in _PROG_CACHE:
        _PROG_CACHE["nc"] = _build_program()
    return _PROG_CACHE["nc"]


def _host_prep(x, freqs_cos, freqs_sin, cache_k, cache_v, wq, wk, wv, wo):
    """Build the 8 per-core input maps (all fp16)."""
    f32 = np.float32
    f16 = np.float16
    x = np.asarray(x, f32)
    cos = np.asarray(freqs_cos, f32).reshape(-1)[:HD // 2]
    sin = np.asarray(freqs_sin, f32).reshape(-1)[:HD // 2]
    wq = np.asarray(wq, f32)
    wk = np.asarray(wk, f32)
    wv = np.asarray(wv, f32)
    wo = np.asarray(wo, f32)
    cache_k = np.asarray(cache_k, f32)
    cache_v = np.asarray(cache_v, f32)

    def rope_fold(w, nheads):
        w4 = w.reshape(nheads, HD // 2, 2, DIM)
        a, bb = w4[:, :, 0, :], w4[:, :, 1, :]
        c = cos[None, :, None]
        s = sin[None, :, None]
        out = np.empty_like(w4)
        out[:, :, 0, :] = a * c - bb * s
        out[:, :, 1, :] = a * s + bb * c
        return out.reshape(nheads * HD, DIM)

    wq_r = rope_fold(wq, NKV * NG) * f32(1.0 / np.sqrt(HD))
    wk_r = rope_fold(wk, NKV)

    x2 = x.reshape(B, DIM)
    xTp = np.ascontiguousarray(
        x2.T.reshape(ND, 128, B).transpose(1, 0, 2)).reshape(128, ND * B)
    xTp = xTp.astype(f16)

    # K transposed per (h, b): [h, b, d, t]
    KT_all = np.ascontiguousarray(
        cache_k.transpose(2, 0, 3, 1)).astype(f16)
    # V chunked per (h, b): [h, b, p, (n d)] with p = t within 128-chunk n
    cv = cache_v.reshape(B, NT, 128, NKV, HD)
    Vp_all = np.ascontiguousarray(
        cv.transpose(3, 0, 2, 1, 4)).reshape(NKV, B, 128, T).astype(f16)

    ones = np.ones((128, 128), f16)

    in_maps = []
    for h in range(N_CORES):
        wqkvT = np.ascontiguousarray(np.concatenate([
            wq_r[h * NG * HD:(h + 1) * NG * HD],
            wk_r[h * HD:(h + 1) * HD],
            wv[h * HD:(h + 1) * HD],
        ], axis=0).T).astype(f16)                       # [4096, 768]
        woT = np.ascontiguousarray(
            wo[:, h * NG * HD:(h + 1) * NG * HD].T).astype(f16)
        in_maps.append({
            "xTp": xTp,
            "wqkvT": wqkvT,
            "woT": woT,
            "KT": KT_all[h],
            "Vp": Vp_all[h],
            "ones": ones,
        })
    return in_maps


def _kernel_numpy_fallback(x, start_pos, freqs_cos, freqs_sin, cache_k, cache_v,
                           wq, wk, wv, wo):
    """Reference-equivalent numpy path for shapes this kernel isn't built for."""
    f32 = np.float32
    start_pos = int(start_pos)
    x = np.asarray(x, f32)
    bsz, seqlen, _ = x.shape
    n_rep = 4
    hd = HD

    def rope(t, c, s):
        tr = t.reshape(*t.shape[:-1], hd // 2, 2)
        a, b2 = tr[..., 0], tr[..., 1]
        c = c[None, :, None, :]
        s = s[None, :, None, :]
        out = np.stack([a * c - b2 * s, a * s + b2 * c], axis=-1)
        return out.reshape(t.shape)

    xq = (x @ np.asarray(wq, f32).T).reshape(bsz, seqlen, NKV * n_rep, hd)
    xk = (x @ np.asarray(wk, f32).T).reshape(bsz, seqlen, NKV, hd)
    xv = (x @ np.asarray(wv, f32).T).reshape(bsz, seqlen, NKV, hd)
    fc = np.asarray(freqs_cos, f32)
    fs = np.asarray(freqs_sin, f32)
    xq = rope(xq, fc, fs)
    xk = rope(xk, fc, fs)
    ck = np.array(cache_k, f32, copy=True)
    cvv = np.array(cache_v, f32, copy=True)
    ck[:, start_pos:start_pos + seqlen] = xk
    cvv[:, start_pos:start_pos + seqlen] = xv
    keys = ck[:, :start_pos + seqlen]
    values = cvv[:, :start_pos + seqlen]
    q = xq.reshape(bsz, seqlen, NKV, n_rep, hd)
    scale = 1.0 / np.sqrt(hd)
    scores = np.einsum('bsgrd,btgd->bgrst', q, keys) * scale
    scores = scores - scores.max(axis=-1, keepdims=True)
    e = np.exp(scores)
    probs = e / e.sum(axis=-1, keepdims=True)
    out = np.einsum('bgrst,btgd->bsgrd', probs, values)
    out = out.reshape(bsz, seqlen, NKV * n_rep * hd)
    return (out @ np.asarray(wo, f32).T).astype(f32)


TRACE = False          # set True (e.g. from test.py) to neuron-profile the run
TRACE_KWARGS = {}
LAST_RESULT = None     # BassKernelResults of the most recent device run


def kernel(x, start_pos, freqs_cos, freqs_sin, cache_k, cache_v, wq, wk, wv, wo):
    global LAST_RESULT
    x = np.asarray(x)
    if (int(start_pos) != T - 1 or x.shape != (B, 1, DIM)
            or np.asarray(cache_k).shape != (B, T, NKV, HD)):
        return _kernel_numpy_fallback(x, start_pos, freqs_cos, freqs_sin,
                                      cache_k, cache_v, wq, wk, wv, wo)

    from concourse.bass_utils import run_bass_kernel_spmd

    nc = _get_program()
    in_maps = _host_prep(x, freqs_cos, freqs_sin, cache_k, cache_v,
                         wq, wk, wv, wo)
    res = run_bass_kernel_spmd(nc, in_maps, list(range(N_CORES)),
                               trace=TRACE, **TRACE_KWARGS)
    LAST_RESULT = res
    out = np.zeros((B, DIM), np.float64)
    for i in range(N_CORES):
        # outT layout [p, (dchunk, b)] -> [B, DIM]
        o = res.results[i]["outT"].reshape(128, 32, B)
        out += o.transpose(2, 1, 0).reshape(B, DIM)
    return out.astype(np.float32).reshape(B, 1, DIM)


# revision 4
# speedup vs baseline: 2.0262x; 1.0553x over previous
"""GQA decode attention (B=32, q_len=1, T=4096, 32 q heads / 8 kv heads, hd=128)
on 8 Trainium2 NeuronCores.

Sharding: tensor-parallel over kv heads - core h owns kv head h (4 q heads),
its slice of wq/wk/wv (ColumnParallel) and wo (RowParallel), and the
cache_k/cache_v slices for that head. Each core computes a partial output
(RowParallel wo) in transposed layout; the host sums the 8 partials.

The kernel is HBM-bandwidth-bound (KV cache streaming), so everything is
fp16 end to end (rel err ~6e-4 vs the fp32 reference, measured on the
actual data):
  - q_len==1 means RoPE is a fixed linear map on the projection outputs, so
    it is folded into wq/wk on the host: w_rot = R(freqs) @ w. The
    1/sqrt(head_dim) score scale is folded into wq too.
  - K cache is stored transposed [hd, t] in fp16: one score matmul per
    128-key tile (K-tile stationary, fast-weight-load path; q streams 4
    columns).
  - V cache is stored [t, d] in fp16 and used stationary in the PV matmul
    (probs stream 4 columns), producing attn directly in [d, g] layout -
    no per-batch transpose.
  - softmax runs unnormalized (exp in fp32 PSUM -> fp16 probs); the
    denominator comes from a ones-column matmul (column sums) + a strided
    DVE reduce, and the normalization uses a ones-matmul broadcast of
    1/sum across partitions.
  - big DMA is split over three hardware rings: K on the sync ring, V on
    the scalar ring, weights/consts on the gpsimd ring, so the 16 DMA
    engines see deeper queues.
"""

import numpy as np

B = 32
DIM = 4096
HD = 128
NKV = 8
NG = 4          # q heads per kv head
T = 4096
NT = 32         # T / 128 key tiles
ND = 32         # DIM / 128 contraction chunks
N_CORES = 8

_PROG_CACHE = {}


def _build_program():
    import concourse.mybir as mybir
    import concourse.tile as tile
    from concourse import bacc

    fp32 = mybir.dt.float32
    f16 = mybir.dt.float16
    af = mybir.ActivationFunctionType
    ax = mybir.AxisListType
    alu = mybir.AluOpType

    nc = bacc.Bacc("TRN2", target_bir_lowering=False, debug=False,
                   num_devices=N_CORES)

    xTp_d = nc.dram_tensor("xTp", [128, ND * B], f16, kind="ExternalInput").ap()
    wqkvT_d = nc.dram_tensor("wqkvT", [DIM, 768], f16, kind="ExternalInput").ap()
    woT_d = nc.dram_tensor("woT", [NG * HD, DIM], f16, kind="ExternalInput").ap()
    KT_d = nc.dram_tensor("KT", [B, HD, T], f16, kind="ExternalInput").ap()
    Vp_d = nc.dram_tensor("Vp", [B, 128, T], f16, kind="ExternalInput").ap()
    ones_d = nc.dram_tensor("ones", [128, 128], f16, kind="ExternalInput").ap()
    # transposed partial output, layout [p, (dchunk, b)]
    out_d = nc.dram_tensor("outT", [128, 32 * B], fp32, kind="ExternalOutput").ap()

    with tile.TileContext(nc) as tc:
        from contextlib import ExitStack
        with ExitStack() as ctx:
            const_pool = ctx.enter_context(tc.tile_pool(name="const", bufs=1))
            wpool = ctx.enter_context(tc.tile_pool(name="w", bufs=6))
            kv_pool = ctx.enter_context(tc.tile_pool(name="kv", bufs=8))
            small = ctx.enter_context(tc.tile_pool(name="small", bufs=2))

            # consts + weights on the gpsimd ring (keeps sync/scalar rings
            # free for the KV stream)
            ones_sb = const_pool.tile([128, 128], f16, name="ones_sb")
            nc.gpsimd.dma_start(ones_sb[:], ones_d[:])
            xTp_sb = const_pool.tile([128, ND * B], f16, name="xTp_sb")
            nc.gpsimd.dma_start(xTp_sb[:], xTp_d[:])
            woT_sb = []
            for g in range(NG):
                t = const_pool.tile([128, DIM], f16, name=f"woT{g}_sb",
                                    tag=f"woT{g}")
                nc.gpsimd.dma_start(t[:], woT_d[128 * g:128 * (g + 1), :])
                woT_sb.append(t)

            # ---- QKV projections: qT[o,b], kT[o,b], v[b,o] ----
            qT_sb = const_pool.tile([128, NG * B], f16, name="qT_sb")
            kT_sb = const_pool.tile([128, B], f16, name="kT_sb")
            v_sb = const_pool.tile([B, HD], f16, name="v_sb")

            with tc.tile_pool(name="ppsum", bufs=1, space="PSUM") as ppsum:
                psq = [ppsum.tile([128, B], fp32, name=f"psq{g}", tag=f"psq{g}")
                       for g in range(NG)]
                psk = ppsum.tile([128, B], fp32, name="psk", tag="psk")
                psv = ppsum.tile([B, HD], fp32, name="psv", tag="psv")
                for n in range(ND):
                    wch = wpool.tile([128, 768], f16, name="wch", tag="wch")
                    # alternate the two main rings for the weight stream
                    eng = nc.sync if (n % 2 == 0) else nc.scalar
                    eng.dma_start(wch[:], wqkvT_d[128 * n:128 * (n + 1), :])
                    xch = xTp_sb[:, B * n:B * (n + 1)]
                    st, sp = (n == 0), (n == ND - 1)
                    for g in range(NG):
                        nc.tensor.matmul(psq[g][:], wch[:, 128 * g:128 * (g + 1)],
                                         xch, start=st, stop=sp)
                    nc.tensor.matmul(psk[:], wch[:, 512:640], xch,
                                     start=st, stop=sp)
                    nc.tensor.matmul(psv[:], xch, wch[:, 640:768],
                                     start=st, stop=sp)
                for g in range(NG):
                    nc.vector.tensor_copy(qT_sb[:, B * g:B * (g + 1)], psq[g][:])
                nc.vector.tensor_copy(kT_sb[:], psk[:])
                nc.vector.tensor_copy(v_sb[:], psv[:])

            spsum = ctx.enter_context(tc.tile_pool(name="spsum", bufs=3, space="PSUM"))
            opsum = ctx.enter_context(tc.tile_pool(name="opsum", bufs=3, space="PSUM"))
            wpsum = ctx.enter_context(tc.tile_pool(name="wpsum", bufs=2, space="PSUM"))

            qT_re = qT_sb.rearrange("p (g b) -> p b g", b=B)
            attnT_sb = const_pool.tile([128, NG * B], f16, name="attnT_sb")
            attnT_re = attnT_sb.rearrange("p (g b) -> p b g", b=B)

            # ---- attention, one batch at a time ----
            for b in range(B):
                K_sb = kv_pool.tile([128, T], f16, name="K_sb", tag="K")
                nc.sync.dma_start(K_sb[:], KT_d[b])
                V_sb = kv_pool.tile([128, T], f16, name="V_sb", tag="V")
                nc.scalar.dma_start(V_sb[:], Vp_d[b])
                # new-token key: overwrite cache column t=4095
                nc.vector.tensor_copy(K_sb[:, T - 1:T], kT_sb[:, b:b + 1])
                # new-token value: overwrite the t=4095 V row (partition 127
                # of the last chunk). Cross-partition move -> tiny DMA.
                nc.gpsimd.dma_start(
                    V_sb[127:128, 128 * (NT - 1):128 * NT],
                    v_sb[b:b + 1, 0:HD])

                qb = qT_re[:, b]  # [128, 4] strided
                psS = spsum.tile([128, NG * NT], fp32, name="psS", tag="psS")
                for n in range(NT):
                    nc.tensor.matmul(psS[:, NG * n:NG * (n + 1)],
                                     K_sb[:, 128 * n:128 * (n + 1)], qb,
                                     start=True, stop=True)
                probs = kv_pool.tile([128, NG * NT], f16, name="probs",
                                     tag="probs")
                for c in range(2):
                    cw = NG * NT // 2
                    nc.scalar.activation(probs[:, cw * c:cw * (c + 1)],
                                         psS[:, cw * c:cw * (c + 1)], af.Exp)

                # one PSUM bank: cols [0,4) = PV out [d, g]; cols [4,8) =
                # broadcast 1/sum; cols [8,136) partition 0 = column sums
                psO = opsum.tile([128, 8 + NG * NT], fp32, name="psO", tag="psO")
                for n in range(NT):
                    nc.tensor.matmul(psO[:, 0:NG],
                                     V_sb[:, 128 * n:128 * (n + 1)],
                                     probs[:, NG * n:NG * (n + 1)],
                                     start=(n == 0), stop=(n == NT - 1))
                nc.tensor.matmul(psO[0:1, 8:8 + NG * NT], ones_sb[:, 0:1],
                                 probs[:], start=True, stop=True)

                sums4 = small.tile([1, NG], fp32, name="sums4", tag="sums4")
                nc.vector.tensor_reduce(
                    sums4[:],
                    psO[0:1, 8:8 + NG * NT].rearrange("p (n g) -> p g n", g=NG),
                    axis=ax.X, op=alu.add)
                recip = small.tile([1, NG], fp32, name="recip", tag="recip")
                nc.vector.reciprocal(recip[:], sums4[:])
                rr = small.tile([128, NG], f16, name="rr", tag="rr")
                nc.vector.memset(rr[:], 0.0)
                nc.vector.tensor_copy(rr[0:1, :], recip[:])
                nc.tensor.matmul(psO[:, NG:2 * NG], ones_sb[:], rr[:],
                                 start=True, stop=True)
                bc_sb = small.tile([128, NG], fp32, name="bc_sb", tag="bc_sb")
                nc.vector.tensor_copy(bc_sb[:], psO[:, NG:2 * NG])
                nc.vector.tensor_mul(attnT_re[:, b], psO[:, 0:NG], bc_sb[:])

            # ---- wo, woT-stationary: outT[dchunk][d, b] ----
            outT_sb = const_pool.tile([128, 32 * B], fp32, name="outT_sb")
            for j in range(32):
                psW = wpsum.tile([128, B], fp32, name="psW", tag="psW")
                for g in range(NG):
                    nc.tensor.matmul(psW[:], woT_sb[g][:, 128 * j:128 * (j + 1)],
                                     attnT_sb[:, B * g:B * (g + 1)],
                                     start=(g == 0), stop=(g == NG - 1))
                nc.vector.tensor_copy(outT_sb[:, B * j:B * (j + 1)], psW[:])
            nc.sync.dma_start(out_d[:], outT_sb[:])

    nc.compile()
    return nc


def _get_program():
    if "nc" not in _PROG_CACHE:
        _PROG_CACHE["nc"] = _build_program()
    return _PROG_CACHE["nc"]


def _host_prep(x, freqs_cos, freqs_sin, cache_k, cache_v, wq, wk, wv, wo):
    """Build the 8 per-core input maps (all fp16)."""
    f32 = np.float32
    f16 = np.float16
    x = np.asarray(x, f32)
    cos = np.asarray(freqs_cos, f32).reshape(-1)[:HD // 2]
    sin = np.asarray(freqs_sin, f32).reshape(-1)[:HD // 2]
    wq = np.asarray(wq, f32)
    wk = np.asarray(wk, f32)
    wv = np.asarray(wv, f32)
    wo = np.asarray(wo, f32)
    cache_k = np.asarray(cache_k, f32)
    cache_v = np.asarray(cache_v, f32)

    def rope_fold(w, nheads):
        w4 = w.reshape(nheads, HD // 2, 2, DIM)
        a, bb = w4[:, :, 0, :], w4[:, :, 1, :]
        c = cos[None, :, None]
        s = sin[None, :, None]
        out = np.empty_like(w4)
        out[:, :, 0, :] = a * c - bb * s
        out[:, :, 1, :] = a * s + bb * c
        return out.reshape(nheads * HD, DIM)

    wq_r = rope_fold(wq, NKV * NG) * f32(1.0 / np.sqrt(HD))
    wk_r = rope_fold(wk, NKV)

    x2 = x.reshape(B, DIM)
    xTp = np.ascontiguousarray(
        x2.T.reshape(ND, 128, B).transpose(1, 0, 2)).reshape(128, ND * B)
    xTp = xTp.astype(f16)

    # K transposed per (h, b): [h, b, d, t]
    KT_all = np.ascontiguousarray(
        cache_k.transpose(2, 0, 3, 1)).astype(f16)
    # V chunked per (h, b): [h, b, p, (n d)] with p = t within 128-chunk n
    cv = cache_v.reshape(B, NT, 128, NKV, HD)
    Vp_all = np.ascontiguousarray(
        cv.transpose(3, 0, 2, 1, 4)).reshape(NKV, B, 128, T).astype(f16)

    ones = np.ones((128, 128), f16)

    in_maps = []
    for h in range(N_CORES):
        wqkvT = np.ascontiguousarray(np.concatenate([
            wq_r[h * NG * HD:(h + 1) * NG * HD],
            wk_r[h * HD:(h + 1) * HD],
            wv[h * HD:(h + 1) * HD],
        ], axis=0).T).astype(f16)                       # [4096, 768]
        woT = np.ascontiguousarray(
            wo[:, h * NG * HD:(h + 1) * NG * HD].T).astype(f16)
        in_maps.append({
            "xTp": xTp,
            "wqkvT": wqkvT,
            "woT": woT,
            "KT": KT_all[h],
            "Vp": Vp_all[h],
            "ones": ones,
        })
    return in_maps


def _kernel_numpy_fallback(x, start_pos, freqs_cos, freqs_sin, cache_k, cache_v,
                           wq, wk, wv, wo):
    """Reference-equivalent numpy path for shapes this kernel isn't built for."""
    f32 = np.float32
    start_pos = int(start_pos)
    x = np.asarray(x, f32)
    bsz, seqlen, _ = x.shape
    n_rep = 4
    hd = HD

    def rope(t, c, s):
        tr = t.reshape(*t.shape[:-1], hd // 2, 2)
        a, b2 = tr[..., 0], tr[..., 1]
        c = c[None, :, None, :]
        s = s[None, :, None, :]
        out = np.stack([a * c - b2 * s, a * s + b2 * c], axis=-1)
        return out.reshape(t.shape)

    xq = (x @ np.asarray(wq, f32).T).reshape(bsz, seqlen, NKV * n_rep, hd)
    xk = (x @ np.asarray(wk, f32).T).reshape(bsz, seqlen, NKV, hd)
    xv = (x @ np.asarray(wv, f32).T).reshape(bsz, seqlen, NKV, hd)
    fc = np.asarray(freqs_cos, f32)
    fs = np.asarray(freqs_sin, f32)
    xq = rope(xq, fc, fs)
    xk = rope(xk, fc, fs)
    ck = np.array(cache_k, f32, copy=True)
    cvv = np.array(cache_v, f32, copy=True)
    ck[:, start_pos:start_pos + seqlen] = xk
    cvv[:, start_pos:start_pos + seqlen] = xv
    keys = ck[:, :start_pos + seqlen]
    values = cvv[:, :start_pos + seqlen]
    q = xq.reshape(bsz, seqlen, NKV, n_rep, hd)
    scale = 1.0 / np.sqrt(hd)
    scores = np.einsum('bsgrd,btgd->bgrst', q, keys) * scale
    scores = scores - scores.max(axis=-1, keepdims=True)
    e = np.exp(scores)
    probs = e / e.sum(axis=-1, keepdims=True)
    out = np.einsum('bgrst,btgd->bsgrd', probs, values)
    out = out.reshape(bsz, seqlen, NKV * n_rep * hd)
    return (out @ np.asarray(wo, f32).T).astype(f32)


TRACE = False          # set True (e.g. from test.py) to neuron-profile the run
TRACE_KWARGS = {}
LAST_RESULT = None     # BassKernelResults of the most recent device run


def kernel(x, start_pos, freqs_cos, freqs_sin, cache_k, cache_v, wq, wk, wv, wo):
    global LAST_RESULT
    x = np.asarray(x)
    if (int(start_pos) != T - 1 or x.shape != (B, 1, DIM)
            or np.asarray(cache_k).shape != (B, T, NKV, HD)):
        return _kernel_numpy_fallback(x, start_pos, freqs_cos, freqs_sin,
                                      cache_k, cache_v, wq, wk, wv, wo)

    from concourse.bass_utils import run_bass_kernel_spmd

    nc = _get_program()
    in_maps = _host_prep(x, freqs_cos, freqs_sin, cache_k, cache_v,
                         wq, wk, wv, wo)
    res = run_bass_kernel_spmd(nc, in_maps, list(range(N_CORES)),
                               trace=TRACE, **TRACE_KWARGS)
    LAST_RESULT = res
    out = np.zeros((B, DIM), np.float64)
    for i in range(N_CORES):
        # outT layout [p, (dchunk, b)] -> [B, DIM]
        o = res.results[i]["outT"].reshape(128, 32, B)
        out += o.transpose(2, 1, 0).reshape(B, DIM)
    return out.astype(np.float32).reshape(B, 1, DIM)
